# revision 18
# baseline (speedup 1.0000x reference)
"""Trainium2 Bass kernel for nn_EnhancedBilinearInteraction.

Computes out[b, m] = sum_l tanh(bn(x)[b,l,m]) * tanh(bn(y)[b,l,m]) where bn is
training-mode batchnorm over (B, L) per feature m (biased variance).

Strategy (8 NeuronCores, data-parallel over B, B_loc = 8 per core):
  - Host supplies each core's shard twice: natural (l-major) layout for the
    stats pass, and an m-major transposed copy for the normalize/product pass
    (feature index on the SBUF partition axis), plus gamma/beta as [128, 2].
  - Pass 1 (stats): stream natural [128, 2048] tiles; ScalarE squares them;
    TensorE ones-matmuls accumulate per-feature sum / sumsq into PSUM
    (partition-axis contraction). Pure f32.
  - 4 KB AllReduce of (sum_x, sumsq_x, sum_y, sumsq_y) across the 8 cores.
  - Scale/bias: s = gamma * rsqrt(var + eps) (Sqrt + exact reciprocal + 2
    Newton refinements), b = beta - mean * s, laid out per-partition [128, 2].
  - Pass 2: stream m-major [128, 4096] tiles; one ScalarE op does
    tanh(s*x + b) in place (per-partition scale/bias); one VectorE
    scalar_tensor_tensor computes xb*yb with accum_out giving the partial
    L-sums directly. Final tiny PE transpose writes out (8, 256) per core.
"""
import numpy as np
from contextlib import ExitStack

import concourse.bass as bass
import concourse.bacc as bacc
import concourse.tile as tile
import concourse.mybir as mybir
from concourse.bass_utils import run_bass_kernel_spmd

F32 = mybir.dt.float32
BF16 = mybir.dt.bfloat16
AF = mybir.ActivationFunctionType
ALU = mybir.AluOpType

N_CORES = 8
B, L, M = 64, 8192, 256
B_LOC = B // N_CORES            # 8
N_TOTAL = float(B * L)          # 524288 elements per feature
EPS = 1e-5

LF1 = 2048                      # pass-1 tile free dim (1 MiB tiles)
NT1 = (B_LOC * L * M) // (128 * LF1)   # 64 tiles per tensor per core
SL1 = LF1 // 512                # 4 matmul slices per tile (fp32 moving max 512)
LF2 = 4096                      # pass-2 tile free dim (2 MiB tiles)
NLT = L // LF2                  # 2 l-tiles per (b, mc)

_NC_CACHE = {}


def _build_nc():
    if "nc" in _NC_CACHE:
        return _NC_CACHE["nc"]
    nc = bacc.Bacc("TRN2", target_bir_lowering=False, debug=False,
                   num_devices=N_CORES)

    x1m = nc.dram_tensor("x1m", [B_LOC, 2, 128, L], BF16, kind="ExternalInput")
    y_nat = nc.dram_tensor("y_nat", [NT1, 128, LF1], BF16, kind="ExternalInput")
    x_t = nc.dram_tensor("x_t", [B_LOC, 2, 128, L], F32, kind="ExternalInput")
    y_t = nc.dram_tensor("y_t", [B_LOC, 2, 128, L], F32, kind="ExternalInput")
    gamma2 = nc.dram_tensor("gamma2", [128, 2], F32, kind="ExternalInput")
    beta2 = nc.dram_tensor("beta2", [128, 2], F32, kind="ExternalInput")
    out_d = nc.dram_tensor("out", [B_LOC, M], F32, kind="ExternalOutput")

    ones_d = nc.inline_tensor(np.ones((128, 1), np.float32), name="ones_c")
    ident_d = nc.inline_tensor(np.eye(128, dtype=np.float32), name="ident_c")

    with tile.TileContext(nc) as tc:
        with ExitStack() as ctx:
            const = ctx.enter_context(tc.tile_pool(name="const", bufs=1))
            p1x = ctx.enter_context(tc.tile_pool(name="p1x", bufs=3))
            p1y = ctx.enter_context(tc.tile_pool(name="p1y", bufs=3))
            p1sq = ctx.enter_context(tc.tile_pool(name="p1sq", bufs=2))
            pstat = ctx.enter_context(tc.tile_pool(name="pstat", bufs=1, space="PSUM"))
            small = ctx.enter_context(tc.tile_pool(name="small", bufs=1))
            dram = ctx.enter_context(tc.tile_pool(name="dramp", bufs=1, space="DRAM"))
            p2x = ctx.enter_context(tc.tile_pool(name="p2x", bufs=4))
            p2y = ctx.enter_context(tc.tile_pool(name="p2y", bufs=3))
            p2pr = ctx.enter_context(tc.tile_pool(name="p2pr", bufs=1))
            pout = ctx.enter_context(tc.tile_pool(name="pout", bufs=1, space="PSUM"))

            ones_sb = const.tile([128, 1], F32)
            nc.gpsimd.dma_start(ones_sb[:], ones_d.ap())
            ones_bf = const.tile([128, 1], BF16)
            nc.gpsimd.dma_start(ones_bf[:], ones_d.ap())  # SWDGE casts f32->bf16
            ident_sb = const.tile([128, 128], F32)
            nc.gpsimd.dma_start(ident_sb[:], ident_d.ap())
            gamma_sb = const.tile([128, 2], F32)
            nc.gpsimd.dma_start(gamma_sb[:], gamma2.ap())
            beta_sb = const.tile([128, 2], F32)
            nc.gpsimd.dma_start(beta_sb[:], beta2.ap())

            # ---- pass 1: per-core per-feature sum and sumsq ----
            # x: m-major bf16 tiles; bn_stats fuses mean+M2 per partition
            #    (= per feature) in one VectorE stream; bn_aggr merges groups.
            # y: natural bf16 tiles; ScalarE square + TensorE ones-matmul
            #    contractions (per-feature sums land in PSUM [1, 512]).
            acc_sum_y = pstat.tile([1, 512], F32)
            acc_sq_y = pstat.tile([1, 512], F32)
            NXT = B_LOC * 2 * (L // LF1)   # 64 x-tiles; 32 per m-chunk
            GRP = LF1 // 512               # bn_stats calls per tile
            bnacc = [small.tile([128, (NXT // 2) * GRP * 6], F32, name=f"bnacc{c}")
                     for c in range(2)]

            def stats_tile_x(b, mc, lt, slot):
                tl = p1x.tile([128, LF1], BF16, name="tx")
                nc.sync.dma_start(tl[:], x1m.ap()[b, mc, :, lt * LF1:(lt + 1) * LF1])
                for k in range(GRP):
                    nc.vector.bn_stats(
                        bnacc[mc][:, (slot * GRP + k) * 6:(slot * GRP + k) * 6 + 6],
                        tl[:, k * 512:(k + 1) * 512])

            def direct_mms(tl_ap, acc, first, last):
                for j in range(SL1):
                    nc.tensor.matmul(
                        acc[:], ones_bf[:], tl_ap[:, j * 512:(j + 1) * 512],
                        start=(first and j == 0), stop=(last and j == SL1 - 1))

            R1 = LF1 // M

            def fold1_mms(eng, tl_ap, acc, first, last):
                # halve rows 8->4 on `eng` (in place), then two N=512 matmuls
                v8 = tl_ap.rearrange("p (r m) -> p r m", r=R1, m=M)
                eng.tensor_tensor(v8[:, 0:4], v8[:, 0:4], v8[:, 4:8], ALU.add)
                for j in range(2):
                    nc.tensor.matmul(
                        acc[:], ones_bf[:], tl_ap[:, j * 512:(j + 1) * 512],
                        start=(first and j == 0), stop=(last and j == 1))

            def stats_tile_y(t):
                tl = p1y.tile([128, LF1], BF16, name="ty")
                nc.sync.dma_start(tl[:], y_nat.ap()[t])
                first, last = t == 0, t == NT1 - 1
                sq = p1sq.tile([128, LF1], BF16, name="sqy")
                nc.scalar.activation(sq[:], tl[:], AF.Square)
                fold1_mms(nc.gpsimd, tl[:], acc_sum_y, first, last)
                fold1_mms(nc.vector, sq[:], acc_sq_y, first, last)

            NLT1 = L // LF1
            for t in range(NT1):
                b, mc, lt = t // (2 * NLT1), (t // NLT1) % 2, t % NLT1
                stats_tile_x(b, mc, lt, (t // (2 * NLT1)) * NLT1 + t % NLT1)
                stats_tile_y(t)

            # local stats, all per-partition: statsL[:, s*2+mc]
            N_LOC = float(B_LOC * L)
            statsL = small.tile([128, 4], F32)
            for mc in range(2):
                mv = small.tile([128, 2], F32, name=f"mv{mc}")
                nc.vector.bn_aggr(mv[:], bnacc[mc][:])
                msq = small.tile([128, 1], F32, name=f"msq_x{mc}")
                nc.vector.tensor_tensor(msq[:], mv[:, 0:1], mv[:, 0:1], ALU.mult)
                nc.vector.tensor_tensor(msq[:], mv[:, 1:2], msq[:], ALU.add)
                nc.vector.tensor_scalar_mul(statsL[:, 2 + mc:3 + mc], msq[:], N_LOC)
                nc.vector.tensor_scalar_mul(statsL[:, mc:mc + 1], mv[:, 0:1], N_LOC)

            bounce_in = dram.tile([128, 8], F32)
            bounce_out = dram.tile([128, 8], F32)
            nc.gpsimd.dma_start(bounce_in[:, 0:4], statsL[:])
            # y accumulators: [1,512] = (r mod 2, m); fold halves -> [1,256]
            # (m = c*128 + p), packed p-major (pos = p*4 + s*2 + c) into a flat
            # row, then bounce via DRAM to scatter across partitions.
            yp = small.tile([1, 512], F32)
            ypv = yp[:].rearrange("a (p s c) -> a s c p", p=128, s=2, c=2)
            for s, acc in enumerate([acc_sum_y, acc_sq_y]):
                tmp512 = small.tile([1, 512], F32, name=f"tmp512_{s}")
                nc.vector.tensor_copy(tmp512[:], acc[:])
                halves = tmp512[:].rearrange("a (r c p) -> r a c p", r=2, c=2, p=128)
                nc.vector.tensor_tensor(ypv[:, s], halves[0], halves[1], ALU.add)
            yscratch = dram.tile([1, 512], F32)
            nc.gpsimd.dma_start(yscratch[:], yp[:])
            nc.gpsimd.dma_start(
                bounce_in[:, 4:8],
                yscratch[:].rearrange("a (p k) -> (a p) k", p=128, k=4))
            nc.gpsimd.collective_compute(
                "AllReduce", ALU.add,
                replica_groups=[list(range(N_CORES))],
                ins=[bounce_in.opt()], outs=[bounce_out.opt()],
            )
            statsT = small.tile([128, 8], F32)
            nc.gpsimd.dma_start(statsT[:], bounce_out[:])

            # ---- stats -> scale/bias, all [128, 2] per-partition ----
            def finalize(k_sum, k_sq):
                mean = small.tile([128, 2], F32, name=f"mean{k_sum}")
                nc.vector.tensor_scalar_mul(mean[:], statsT[:, k_sum:k_sum + 2], 1.0 / N_TOTAL)
                veps = small.tile([128, 2], F32, name=f"veps{k_sum}")
                nc.vector.tensor_scalar_mul(veps[:], statsT[:, k_sq:k_sq + 2], 1.0 / N_TOTAL)
                msq = small.tile([128, 2], F32, name=f"msq{k_sum}")
                nc.vector.tensor_tensor(msq[:], mean[:], mean[:], ALU.mult)
                nc.vector.tensor_tensor(veps[:], veps[:], msq[:], ALU.subtract)
                nc.vector.tensor_scalar_add(veps[:], veps[:], EPS)
                sq = small.tile([128, 2], F32, name=f"sqv{k_sum}")
                nc.scalar.activation(sq[:], veps[:], AF.Sqrt)
                r = small.tile([128, 2], F32, name=f"r{k_sum}")
                nc.vector.reciprocal(r[:], sq[:])
                tmp = small.tile([128, 2], F32, name=f"tmp{k_sum}")
                for _ in range(2):  # Newton rsqrt refinement (Sqrt table is loose)
                    nc.vector.tensor_tensor(tmp[:], r[:], r[:], ALU.mult)
                    nc.vector.tensor_tensor(tmp[:], tmp[:], veps[:], ALU.mult)
                    nc.vector.tensor_scalar(tmp[:], tmp[:], -0.5, 1.5, ALU.mult, ALU.add)
                    nc.vector.tensor_tensor(r[:], r[:], tmp[:], ALU.mult)
                s_t = small.tile([128, 2], F32, name=f"s{k_sum}")
                nc.vector.tensor_tensor(s_t[:], gamma_sb[:], r[:], ALU.mult)
                b_t = small.tile([128, 2], F32, name=f"b{k_sum}")
                nc.vector.tensor_tensor(b_t[:], mean[:], s_t[:], ALU.mult)
                nc.vector.tensor_tensor(b_t[:], beta_sb[:], b_t[:], ALU.subtract)
                return s_t, b_t

            s_x, b_x = finalize(0, 2)
            s_y, b_y = finalize(4, 6)

            # ---- pass 2: tanh-normalize, product, L-reduction ----
            acc = small.tile([128, B_LOC * 2 * NLT], F32)
            for b in range(B_LOC):
                for mc in range(2):
                    for lt in range(NLT):
                        xt2 = p2x.tile([128, LF2], F32, name="xt2")
                        nc.sync.dma_start(
                            xt2[:], x_t.ap()[b, mc, :, lt * LF2:(lt + 1) * LF2])
                        yt2 = p2y.tile([128, LF2], F32, name="yt2")
                        nc.scalar.dma_start(
                            yt2[:], y_t.ap()[b, mc, :, lt * LF2:(lt + 1) * LF2])
                        nc.scalar.activation(
                            xt2[:], xt2[:], AF.Tanh,
                            bias=b_x[:, mc:mc + 1], scale=s_x[:, mc:mc + 1])
                        nc.scalar.activation(
                            yt2[:], yt2[:], AF.Tanh,
                            bias=b_y[:, mc:mc + 1], scale=s_y[:, mc:mc + 1])
                        col = (b * 2 + mc) * NLT + lt
                        prod = p2pr.tile([128, LF2], BF16, name="prod")
                        nc.vector.scalar_tensor_tensor(
                            prod[:], xt2[:], 1.0, yt2[:], ALU.mult, ALU.mult,
                            accum_out=acc[:, col:col + 1])

            red = small.tile([128, B_LOC * 2], F32)
            nc.vector.tensor_reduce(
                red[:], acc[:].rearrange("p (g lt) -> p g lt", lt=NLT),
                axis=mybir.AxisListType.X, op=ALU.add)
            outp = pout.tile([16, 128], F32)
            nc.tensor.transpose(outp[:], red[:], ident_sb[:])
            out_sb = small.tile([16, 128], F32)
            nc.vector.tensor_copy(out_sb[:], outp[:])
            nc.gpsimd.dma_start(
                out_d.ap().rearrange("b (mc p) -> (b mc) p", mc=2), out_sb[:])

    nc.compile()
    _NC_CACHE["nc"] = nc
    return nc


def make_in_maps(inputs):
    import ml_dtypes
    bf16 = np.dtype(ml_dtypes.bfloat16)
    x = np.ascontiguousarray(np.asarray(inputs["x"], dtype=np.float32))
    y = np.ascontiguousarray(np.asarray(inputs["y"], dtype=np.float32))
    gamma2 = np.ascontiguousarray(
        np.asarray(inputs["gamma"], dtype=np.float32).reshape(2, 128).T)
    beta2 = np.ascontiguousarray(
        np.asarray(inputs["beta"], dtype=np.float32).reshape(2, 128).T)
    in_maps = []
    for c in range(N_CORES):
        xs = x[c * B_LOC:(c + 1) * B_LOC]
        ys = y[c * B_LOC:(c + 1) * B_LOC]
        x_t = np.ascontiguousarray(xs.transpose(0, 2, 1)).reshape(B_LOC, 2, 128, L)
        in_maps.append({
            "x1m": x_t.astype(bf16),
            "y_nat": ys.reshape(NT1, 128, LF1).astype(bf16),
            "x_t": x_t,
            "y_t": np.ascontiguousarray(ys.transpose(0, 2, 1)).reshape(B_LOC, 2, 128, L),
            "gamma2": gamma2,
            "beta2": beta2,
        })
    return in_maps


def kernel(x, y, gamma, beta):
    nc = _build_nc()
    in_maps = make_in_maps({"x": x, "y": y, "gamma": gamma, "beta": beta})
    res = run_bass_kernel_spmd(nc, in_maps, core_ids=list(range(N_CORES)))
    return np.concatenate([res.results[c]["out"] for c in range(N_CORES)], axis=0)
